# revision 22
# baseline (speedup 1.0000x reference)
"""Causal self-attention on 8 Trainium2 NeuronCores.

Sharding (data + head parallel): core c handles batch b = c // 4 and the
4 heads [4g, 4g+4) where g = c % 4.  Each core projects q/k/v for its
heads (weights pre-sliced + pre-transposed on host), runs causal
attention, then the 4 cores of each batch AllGather the per-head
attention outputs (hd-major fp16) and each computes a disjoint
256-channel column slice of the output projection.

Schedule notes (v4):
- The attention inner loop is exp(ACT)-paced (~1us per 128-key step);
  projection work (q/k second halves, v tiles, output projection) is
  emitted as small self-contained "filler" closures drained one per
  j-step inside the attention loops, so the in-order PE queue never
  parks a multi-us block in front of the next score matmul.
- One AllGather per 512-column chunk (both head pairs) fires as soon as
  the chunk is staged on both pairs; gather g's SBUF prefetch is issued
  about one chunk later, and its out_proj slice runs as filler inside a
  later chunk.  Only the final chunk's gather + out_proj sit in the
  tail.
- Dummy matmuls on the first-loaded weight tile warm the PE clock (HAM
  K=8/8) while x streams in, and again while the last gather flies so
  the tail out_proj runs at 2.4GHz.
- Softmax normalization: accumulators leave PSUM in one copy (frees the
  at/pp space immediately); the 1024 denominators are reshaped across
  64 partitions by DMA, inverted (DVE reciprocal is serial per
  partition), reshaped back, broadcast once to [64,1024], and applied
  with a single multiply; one DMA ships both heads to the collective
  buffer.
- A tiny warmup AllGather at kernel start absorbs CC-stream ramp and
  cross-core launch skew.
- fp16 data path, fp32 PSUM accumulation, fp16 output (absmax ~4).
- PSUM budget (8 banks): score tile 2 banks x 2 bufs; "at"/"pp" spaces
  2 banks each, alternating between attention accumulators and
  filler projection tiles (fillers always use the opposite space of
  the chunk they are drained into).

Layouts per core:
  xT    (1024, 2048)  x[b].T                       (d on partitions)
  wqkT  (1024, 512)   [ (Wq[rows]/8).T | Wk[rows].T ]
  wvT   (1024, 256)   Wv[rows].T
  woT   (1024, 256)   Wo[rows].T with rows permuted to the AllGather
                      order: [pair p=0: rank r: heads 4r,4r+1] then
                      [pair p=1: rank r: heads 4r+2,4r+3]
  mask  (128, 128)    upper-triangular ones (k <= q)
  outT  (256, 2048)   out[b][:, cols].T  (fp16)
"""

from collections import deque

import numpy as np

B, S, D, H = 2, 2048, 1024, 16
HD = D // H              # 64
NCORES = 8
GROUP = 4                # cores per batch
LHEADS = 4               # heads per core
LCH = LHEADS * HD        # 256 local channels
KT = D // 128            # 8 contraction tiles
ST = S // 128            # 16 sequence tiles
PAIRS = 2                # head pairs per core
CHUNK = 512              # q columns per attention pass / gather block
NCH = S // CHUNK         # 4

_CACHE = {}


def _f16(a):
    return np.ascontiguousarray(a, dtype=np.float16)


def _build():
    import concourse.bacc as bacc
    import concourse.mybir as mybir
    import concourse.tile as tile

    f32 = mybir.dt.float32
    f16 = mybir.dt.float16
    Exp = mybir.ActivationFunctionType.Exp

    nc = bacc.Bacc(num_devices=NCORES)
    xT = nc.dram_tensor("xT", [D, S], f16, kind="ExternalInput")
    wqkT = nc.dram_tensor("wqkT", [D, 2 * LCH], f16, kind="ExternalInput")
    wvT = nc.dram_tensor("wvT", [D, LCH], f16, kind="ExternalInput")
    woT = nc.dram_tensor("woT", [D, LCH], f16, kind="ExternalInput")
    mask = nc.dram_tensor("mask", [128, 128], f16, kind="ExternalInput")
    outT = nc.dram_tensor("outT", [LCH, S], f16, kind="ExternalOutput")

    RG = [[0, 1, 2, 3], [4, 5, 6, 7]]

    with tile.TileContext(nc, num_cores=NCORES) as tc:
        with (
            tc.tile_pool(name="const", bufs=1) as const,
            tc.tile_pool(name="qkv", bufs=1) as qkv,
            tc.tile_pool(name="psum", bufs=1, space="PSUM") as psum,
            tc.tile_pool(name="dram", bufs=1, space="DRAM") as dram,
            tc.tile_pool(name="work", bufs=1) as work,
            tc.tile_pool(name="proj", bufs=1) as projp,
            tc.tile_pool(name="agp", bufs=1) as agp,
        ):
            # warmup collective first: absorbs CC-stream ramp + launch skew
            warm_in = dram.tile([128, 8], f16, name="warm_in")
            warm_out = dram.tile([GROUP * 128, 8], f16, name="warm_out")
            nc.sync.dma_start(warm_in[:], mask[:, 0:8])
            nc.gpsimd.collective_compute(
                "AllGather", mybir.AluOpType.bypass, replica_groups=RG,
                ins=[warm_in[:]], outs=[warm_out[:]])

            # chunks 0-2 gather both pairs at once; chunk 3 gathers per pair
            # so the tail only waits on the last 128KB collective
            cc_in = [dram.tile([128, 2 * CHUNK], f16, name=f"ccin{g}")
                     for g in range(NCH - 1)]
            cc_out = [dram.tile([GROUP * 128, 2 * CHUNK], f16, name=f"ccout{g}")
                      for g in range(NCH - 1)]
            cc_in3 = [dram.tile([128, CHUNK], f16, name=f"ccin3{p}")
                      for p in range(PAIRS)]
            cc_out3 = [dram.tile([GROUP * 128, CHUNK], f16, name=f"ccout3{p}")
                       for p in range(PAIRS)]

            mask_sb = const.tile([128, 128], f16)
            ones4 = const.tile([128, LHEADS], f32)
            nc.vector.memset(ones4[:], 1.0)

            qt = qkv.tile([128, PAIRS, S], f16)
            kt = qkv.tile([128, PAIRS, S], f16)
            v = qkv.tile([128, ST, LHEADS, 65], f16)

            # ---------------- input loads ----------------
            # spread across the two hardware DGE rings (sync + scalar) so
            # per-ring serialization + coarse semaphores don't delay the
            # first projection matmuls
            wqk = projp.tile([128, KT, 2 * LCH], f16)
            nc.scalar.dma_start(wqk[:], wqkT[:].rearrange("(k p) n -> p k n", p=128))
            nc.sync.dma_start(mask_sb[:], mask[:])
            xt = []
            for k in range(KT):
                tx = projp.tile([128, S], f16, name=f"xt{k}")
                eng = nc.sync if k % 2 == 0 else nc.scalar
                eng.dma_start(tx[:], xT[128 * k:128 * k + 128, :])
                xt.append(tx)
            wv = projp.tile([128, KT, LCH], f16)
            nc.sync.dma_start(wv[:], wvT[:].rearrange("(k p) n -> p k n", p=128))
            wo = projp.tile([128, KT, LCH], f16)
            nc.scalar.dma_start(wo[:], woT[:].rearrange("(k p) n -> p k n", p=128))

            def warm_pe(n, name, tag):
                """Dummy matmuls on the wqk tile: keep the PE HAM warm while
                it would otherwise idle (startup x-load, tail gather wait)."""
                junk = psum.tile([128, 512], f32, tag=tag, name=name)
                for i in range(n):
                    nc.tensor.matmul(
                        junk[:], wqk[:, 0, 0:128], wqk[:, i % KT, 0:512],
                        start=True, stop=True)

            warm_pe(12, "warmup", "at")

            def qk_half(m, half, tag):
                # m: 0,1 = q pair 0/1; 2,3 = k pair 0/1; half = 1024 cols
                dst = qt if m < 2 else kt
                pp = psum.tile([128, 1024], f32, tag=tag, name=f"qk{m}{half}")
                for k in range(KT):
                    for c2 in range(2):
                        o = 1024 * half + 512 * c2
                        nc.tensor.matmul(
                            pp[:, 512 * c2:512 * c2 + 512],
                            wqk[:, k, 128 * m:128 * m + 128],
                            xt[k][:, o:o + 512],
                            start=(k == 0), stop=(k == KT - 1))
                nc.vector.tensor_copy(
                    dst[:, m % 2, 1024 * half:1024 * half + 1024], pp[:])

            def v_proj(j, tag):
                vps = psum.tile([128, LCH], f32, tag=tag, name=f"v{j}")
                for k in range(KT):
                    nc.tensor.matmul(
                        vps[:], xt[k][:, 128 * j:128 * j + 128], wv[:, k, :],
                        start=(k == 0), stop=(k == KT - 1))
                nc.vector.tensor_copy(
                    v[:, j, :, 64:65], ones4[:].rearrange("p (h o) -> p h o", o=1))
                nc.vector.tensor_copy(
                    v[:, j, :, 0:64], vps[:].rearrange("p (h e) -> p h e", h=LHEADS))

            ag = {}
            ccin_last = [None]

            def stage_chunk(p, c, attps):
                """Copy the accumulators out of PSUM in one shot (frees the
                at/pp space immediately), then normalize from SBUF and ship
                both heads to the collective buffer in one DMA."""
                asb = work.tile([65, 2 * CHUNK], f32, tag="asb", bufs=2,
                                name=f"asb{p}{c}")
                nc.vector.tensor_copy(asb[:], attps[:])
                rcs = work.tile([64, 16], f32, tag="rcs", bufs=2,
                                name=f"rcs{p}{c}")
                nc.sync.dma_start(rcs[:], asb[64:65, :])
                rcr = work.tile([64, 16], f32, tag="rcr", bufs=2,
                                name=f"rcr{p}{c}")
                nc.vector.reciprocal(rcr[:], rcs[:])
                rc0 = work.tile([1, 2 * CHUNK], f32, tag="rc0", bufs=2,
                                name=f"rc0{p}{c}")
                nc.sync.dma_start(rc0[0:1, :], rcr[:])
                bc = work.tile([64, 2 * CHUNK], f32, tag="bc", bufs=2,
                               name=f"bc{p}{c}")
                nc.gpsimd.partition_broadcast(bc[:], rc0[0:1, :])
                ao = work.tile([64, 2 * CHUNK], f16, tag="ao", bufs=2,
                               name=f"ao{p}{c}")
                nc.vector.tensor_mul(ao[:, :], asb[0:64, :], bc[:, :])
                # both heads in one DMA: src col 512h+i -> dst row 64h+q
                if c == 3:
                    dst = cc_in3[p][:].rearrange("(h q) n -> q h n", h=2)
                    ccin_last[0] = nc.sync.dma_start(dst[:, :, :], ao[:, :])
                else:
                    dst = cc_in[c][:].rearrange("(h q) n -> q h n", h=2)
                    ccin_last[0] = nc.sync.dma_start(
                        dst[:, :, CHUNK * p:CHUNK * p + CHUNK], ao[:, :])

            def gather(g):
                nc.gpsimd.collective_compute(
                    "AllGather", mybir.AluOpType.bypass, replica_groups=RG,
                    ins=[cc_in[g][:]], outs=[cc_out[g][:]])

            def gather3(p):
                nc.gpsimd.collective_compute(
                    "AllGather", mybir.AluOpType.bypass, replica_groups=RG,
                    ins=[cc_in3[p][:]], outs=[cc_out3[p][:]])

            def prefetch(g):
                # Pin behind the newest staging DMA so the scheduler cannot
                # hoist the AllGather wait ahead of attention staging.
                for r in range(GROUP):
                    t = agp.tile([128, 2 * CHUNK], f16, name=f"ag{g}{r}")
                    dma = nc.sync.dma_start(
                        t[:], cc_out[g][128 * r:128 * r + 128, :])
                    if ccin_last[0] is not None:
                        tile.add_dep_helper(
                            dma.ins, ccin_last[0].ins, sync=True,
                            reason="gather prefetch after staging")
                    ag[(g, r)] = t

            def prefetch3(p):
                for r in range(GROUP):
                    t = agp.tile([128, CHUNK], f16, name=f"ag3{p}{r}")
                    dma = nc.sync.dma_start(
                        t[:], cc_out3[p][128 * r:128 * r + 128, :])
                    if ccin_last[0] is not None:
                        tile.add_dep_helper(
                            dma.ins, ccin_last[0].ins, sync=True,
                            reason="gather prefetch after staging")
                    ag[(3, p, r)] = t

            def out_ct(g, ct, tag):
                pp = psum.tile([128, CHUNK], f32, tag=tag, name=f"op{g}{ct}")
                for k in range(KT):
                    rhs = (ag[(3, k // 4, k % 4)][:, :] if g == 3 else
                           ag[(g, k % 4)][:, CHUNK * (k // 4):
                                          CHUNK * (k // 4) + CHUNK])
                    nc.tensor.matmul(
                        pp[:], wo[:, k, 128 * ct:128 * ct + 128], rhs,
                        start=(k == 0), stop=(k == KT - 1))
                ot = agp.tile([128, CHUNK], f16, tag=f"ot{ct}", bufs=2,
                              name=f"ot{g}{ct}")
                nc.scalar.copy(ot[:], pp[:])
                nc.sync.dma_start(
                    outT[128 * ct:128 * ct + 128,
                         CHUNK * g:CHUNK * g + CHUNK], ot[:])

            # Filler queue: self-contained closures (own PSUM tile, tag
            # passed at drain time = opposite of the running chunk's attps).
            FILL = deque()

            def drain_all():
                while FILL:
                    FILL.popleft()("pp")   # between chunks: either tag works

            def attn_chunk(p, c, gc):
                opp = "pp" if gc % 2 == 0 else "at"
                q0 = CHUNK * c
                nj = 4 * c + 4
                attps = psum.tile([65, 2 * CHUNK], f32,
                                  tag=("at" if gc % 2 == 0 else "pp"),
                                  name=f"att{p}{c}")
                for j in range(nj):
                    qs = max(q0, 128 * j)
                    n = q0 + CHUNK - qs
                    off = qs - q0
                    sc = psum.tile([128, 1024], f32, tag="sc", bufs=2,
                                   name=f"sc{p}{c}{j}")
                    for h in range(2):
                        pb = 64 * h
                        nc.tensor.matmul(
                            sc[:, 512 * h:512 * h + n],
                            kt[pb:pb + 64, p, 128 * j:128 * j + 128],
                            qt[pb:pb + 64, p, qs:qs + n],
                            start=True, stop=True)
                    ex = work.tile([128, 1024], f16, tag="ex", bufs=3,
                                   name=f"ex{p}{c}{j}")
                    if n == CHUNK:
                        nc.scalar.activation(ex[:, :], sc[:, :], Exp)
                    else:
                        nc.scalar.activation(
                            ex[:].rearrange("q (t x) -> q t x", t=2)[:, :, 0:n],
                            sc[:].rearrange("q (t x) -> q t x", t=2)[:, :, 0:n],
                            Exp)
                    if qs == 128 * j:  # diagonal tile: causal mask
                        for h in range(2):
                            nc.vector.tensor_mul(
                                ex[:, 512 * h:512 * h + 128],
                                ex[:, 512 * h:512 * h + 128], mask_sb[:])
                    for h in range(2):
                        nc.tensor.matmul(
                            attps[:, 512 * h + off:512 * h + CHUNK],
                            v[:, j, 2 * p + h, :],
                            ex[:, 512 * h:512 * h + n],
                            start=(j == 0), stop=(j == nj - 1))
                    if FILL and j < nj - 1 and (j >= 3 or (nj <= 6 and j >= 1)):
                        FILL.popleft()(opp)
                stage_chunk(p, c, attps)

            def qk_fillers(m, half):
                """q/k half as two filler closures sharing one PSUM group;
                drained on consecutive j-slots of a single chunk."""
                state = {}

                def part(ks):
                    def go(tag):
                        if "pp" not in state:
                            state["pp"] = psum.tile(
                                [128, 1024], f32, tag=tag, name=f"qk{m}{half}")
                        pp = state["pp"]
                        for k in ks:
                            for c2 in range(2):
                                o = 1024 * half + 512 * c2
                                nc.tensor.matmul(
                                    pp[:, 512 * c2:512 * c2 + 512],
                                    wqk[:, k, 128 * m:128 * m + 128],
                                    xt[k][:, o:o + 512],
                                    start=(k == 0), stop=(k == KT - 1))
                        if ks[-1] == KT - 1:
                            dst = qt if m < 2 else kt
                            nc.vector.tensor_copy(
                                dst[:, m % 2, 1024 * half:1024 * half + 1024],
                                pp[:])
                    return go
                return [part(range(0, 4)), part(range(4, 8))]

            # ---------------- schedule ----------------
            qk_half(0, 0, "at")
            qk_half(2, 0, "pp")
            v_proj(0, "at")
            v_proj(1, "pp")
            v_proj(2, "at")
            v_proj(3, "pp")
            for j in range(4, 8):
                FILL.append(lambda t, j=j: v_proj(j, t))
            attn_chunk(0, 0, 0)        # attps at
            qk_half(1, 0, "pp")
            qk_half(3, 0, "pp")
            attn_chunk(1, 0, 1)        # attps pp
            gather(0)
            drain_all()                # v4-7 must exist before chunk 1
            FILL.extend(qk_fillers(0, 1))   # 4 closures -> all inside gc2
            FILL.extend(qk_fillers(2, 1))
            for j in range(8, 12):          # 4 closures -> all inside gc3
                FILL.append(lambda t, j=j: v_proj(j, t))
            attn_chunk(0, 1, 2)        # at
            attn_chunk(1, 1, 3)        # pp
            gather(1)
            prefetch(0)
            drain_all()                # q/k half1 (pair 0) + v8-11 ready
            FILL.extend(qk_fillers(1, 1))
            FILL.extend(qk_fillers(3, 1))
            attn_chunk(0, 2, 4)        # at
            drain_all()                # q/k half1 (pair 1) before chunk(1,2)
            for j in range(12, 16):
                FILL.append(lambda t, j=j: v_proj(j, t))
            FILL.append(lambda t: out_ct(0, 0, t))
            FILL.append(lambda t: out_ct(0, 1, t))
            attn_chunk(1, 2, 5)        # pp
            gather(2)
            prefetch(1)
            drain_all()                # v12-15 before chunk 3
            FILL.append(lambda t: out_ct(1, 0, t))
            FILL.append(lambda t: out_ct(1, 1, t))
            attn_chunk(0, 3, 6)        # at
            gather3(0)
            prefetch(2)
            attn_chunk(1, 3, 7)        # pp
            gather3(1)
            drain_all()
            out_ct(2, 0, "at")         # AG2 landed during chunk 3
            out_ct(2, 1, "at")
            warm_pe(24, "tailwarm", "pp")  # keep PE warm while gather3 flies
            prefetch3(0)
            prefetch3(1)
            out_ct(3, 0, "pp")
            out_ct(3, 1, "at")

    nc.compile()
    return nc


def _gather_perm():
    """d-channel permutation matching the AllGather layout."""
    perm = []
    for p in range(PAIRS):
        for r in range(GROUP):
            for h in range(2):
                head = 4 * r + 2 * p + h
                perm.extend(range(HD * head, HD * head + HD))
    return np.array(perm)


def _shard_inputs(x, Wq, Wk, Wv, Wo):
    x = np.asarray(x, dtype=np.float32)
    Wq = np.asarray(Wq, dtype=np.float32)
    Wk = np.asarray(Wk, dtype=np.float32)
    Wv = np.asarray(Wv, dtype=np.float32)
    Wo = np.asarray(Wo, dtype=np.float32)
    mask = np.triu(np.ones((128, 128), dtype=np.float16))
    perm = _gather_perm()
    in_maps = []
    for c in range(NCORES):
        b, g = c // GROUP, c % GROUP
        rows = slice(LCH * g, LCH * g + LCH)
        in_maps.append({
            "xT": _f16(x[b].T),
            "wqkT": _f16(np.concatenate([Wq[rows] / 8.0, Wk[rows]], axis=0).T),
            "wvT": _f16(Wv[rows].T),
            "woT": _f16(Wo[rows].T[perm, :]),
            "mask": mask,
        })
    return in_maps


def kernel(x, Wq, Wk, Wv, Wo):
    from concourse.bass_utils import run_bass_kernel_spmd

    if "nc" not in _CACHE:
        _CACHE["nc"] = _build()
    nc = _CACHE["nc"]
    in_maps = _shard_inputs(x, Wq, Wk, Wv, Wo)
    res = run_bass_kernel_spmd(nc, in_maps, core_ids=list(range(NCORES)))
    _CACHE["last_results"] = res
    out = np.empty((B, S, D), dtype=np.float32)
    for c in range(NCORES):
        b, g = c // GROUP, c % GROUP
        out[b][:, LCH * g:LCH * g + LCH] = \
            res.results[c]["outT"].T.astype(np.float32)
    return out


# revision 24
# speedup vs baseline: 1.0988x; 1.0988x over previous
"""Causal self-attention on 8 Trainium2 NeuronCores.

Sharding (data + head parallel): core c handles batch b = c // 4 and the
4 heads [4g, 4g+4) where g = c % 4.  Each core projects q/k/v for its
heads (weights pre-sliced + pre-transposed on host), runs causal
attention, then the 4 cores of each batch AllGather the per-head
attention outputs (hd-major fp16) and each computes a disjoint
256-channel column slice of the output projection.

Schedule notes (v4):
- The attention inner loop is exp(ACT)-paced (~1us per 128-key step);
  projection work (q/k second halves, v tiles, output projection) is
  emitted as small self-contained "filler" closures drained one per
  j-step inside the attention loops, so the in-order PE queue never
  parks a multi-us block in front of the next score matmul.
- One AllGather per 512-column chunk (both head pairs) fires as soon as
  the chunk is staged on both pairs; gather g's SBUF prefetch is issued
  about one chunk later, and its out_proj slice runs as filler inside a
  later chunk.  Only the final chunk's gather + out_proj sit in the
  tail.
- Dummy matmuls on the first-loaded weight tile warm the PE clock (HAM
  K=8/8) while x streams in, and again while the last gather flies so
  the tail out_proj runs at 2.4GHz.
- Softmax normalization: accumulators leave PSUM in one copy (frees the
  at/pp space immediately); the 1024 denominators are reshaped across
  64 partitions by DMA, inverted (DVE reciprocal is serial per
  partition), reshaped back, broadcast once to [64,1024], and applied
  with a single multiply; one DMA ships both heads to the collective
  buffer.
- A tiny warmup AllGather at kernel start absorbs CC-stream ramp and
  cross-core launch skew.
- fp16 data path, fp32 PSUM accumulation, fp16 output (absmax ~4).
- PSUM budget (8 banks): score tile 2 banks x 2 bufs; "at"/"pp" spaces
  2 banks each, alternating between attention accumulators and
  filler projection tiles (fillers always use the opposite space of
  the chunk they are drained into).

Layouts per core:
  xT    (1024, 2048)  x[b].T                       (d on partitions)
  wqkT  (1024, 512)   [ (Wq[rows]/8).T | Wk[rows].T ]
  wvT   (1024, 256)   Wv[rows].T
  woT   (1024, 256)   Wo[rows].T with rows permuted to the AllGather
                      order: [pair p=0: rank r: heads 4r,4r+1] then
                      [pair p=1: rank r: heads 4r+2,4r+3]
  mask  (128, 128)    upper-triangular ones (k <= q)
  outT  (256, 2048)   out[b][:, cols].T  (fp16)
"""

from collections import deque

import numpy as np

B, S, D, H = 2, 2048, 1024, 16
HD = D // H              # 64
NCORES = 8
GROUP = 4                # cores per batch
LHEADS = 4               # heads per core
LCH = LHEADS * HD        # 256 local channels
KT = D // 128            # 8 contraction tiles
ST = S // 128            # 16 sequence tiles
PAIRS = 2                # head pairs per core
CHUNK = 512              # q columns per attention pass / gather block
NCH = S // CHUNK         # 4

_CACHE = {}


def _f16(a):
    return np.ascontiguousarray(a, dtype=np.float16)


def _build():
    import concourse.bacc as bacc
    import concourse.mybir as mybir
    import concourse.tile as tile

    f32 = mybir.dt.float32
    f16 = mybir.dt.float16
    Exp = mybir.ActivationFunctionType.Exp

    nc = bacc.Bacc(num_devices=NCORES)
    xT = nc.dram_tensor("xT", [D, S], f16, kind="ExternalInput")
    wqkT = nc.dram_tensor("wqkT", [D, 2 * LCH], f16, kind="ExternalInput")
    wvT = nc.dram_tensor("wvT", [D, LCH], f16, kind="ExternalInput")
    woT = nc.dram_tensor("woT", [D, LCH], f16, kind="ExternalInput")
    mask = nc.dram_tensor("mask", [128, 128], f16, kind="ExternalInput")
    outT = nc.dram_tensor("outT", [LCH, S], f16, kind="ExternalOutput")

    RG = [[0, 1, 2, 3], [4, 5, 6, 7]]

    with tile.TileContext(nc, num_cores=NCORES) as tc:
        with (
            tc.tile_pool(name="const", bufs=1) as const,
            tc.tile_pool(name="qkv", bufs=1) as qkv,
            tc.tile_pool(name="psum", bufs=1, space="PSUM") as psum,
            tc.tile_pool(name="dram", bufs=1, space="DRAM") as dram,
            tc.tile_pool(name="work", bufs=1) as work,
            tc.tile_pool(name="proj", bufs=1) as projp,
            tc.tile_pool(name="agp", bufs=1) as agp,
        ):
            # warmup collective first: absorbs CC-stream ramp + launch skew
            warm_in = dram.tile([128, 8], f16, name="warm_in")
            warm_out = dram.tile([GROUP * 128, 8], f16, name="warm_out")
            nc.sync.dma_start(warm_in[:], mask[:, 0:8])
            nc.gpsimd.collective_compute(
                "AllGather", mybir.AluOpType.bypass, replica_groups=RG,
                ins=[warm_in[:]], outs=[warm_out[:]])

            # chunks 0-2 gather both pairs at once; chunk 3 gathers per pair
            # so the tail only waits on the last 128KB collective
            cc_in = [dram.tile([128, 2 * CHUNK], f16, name=f"ccin{g}")
                     for g in range(NCH - 1)]
            cc_out = [dram.tile([GROUP * 128, 2 * CHUNK], f16, name=f"ccout{g}")
                      for g in range(NCH - 1)]
            cc_in3 = [dram.tile([128, CHUNK], f16, name=f"ccin3{p}")
                      for p in range(PAIRS)]
            cc_out3 = [dram.tile([GROUP * 128, CHUNK], f16, name=f"ccout3{p}")
                       for p in range(PAIRS)]

            mask_sb = const.tile([128, 128], f16)
            ones4 = const.tile([128, LHEADS], f32)
            nc.vector.memset(ones4[:], 1.0)

            qt = qkv.tile([128, PAIRS, S], f16)
            kt = qkv.tile([128, PAIRS, S], f16)
            v = qkv.tile([128, ST, LHEADS, 65], f16)

            # ---------------- input loads ----------------
            # spread across the two hardware DGE rings (sync + scalar) so
            # per-ring serialization + coarse semaphores don't delay the
            # first projection matmuls
            wqk = projp.tile([128, KT, 2 * LCH], f16)
            nc.scalar.dma_start(wqk[:], wqkT[:].rearrange("(k p) n -> p k n", p=128))
            nc.sync.dma_start(mask_sb[:], mask[:])
            xt = []
            for k in range(KT):
                tx = projp.tile([128, S], f16, name=f"xt{k}")
                eng = nc.sync if k % 2 == 0 else nc.scalar
                eng.dma_start(tx[:], xT[128 * k:128 * k + 128, :])
                xt.append(tx)
            wv = projp.tile([128, KT, LCH], f16)
            nc.sync.dma_start(wv[:], wvT[:].rearrange("(k p) n -> p k n", p=128))
            wo = projp.tile([128, KT, LCH], f16)
            nc.scalar.dma_start(wo[:], woT[:].rearrange("(k p) n -> p k n", p=128))

            def warm_pe(n, name, tag):
                """Dummy matmuls on the wqk tile: keep the PE HAM warm while
                it would otherwise idle (startup x-load, tail gather wait)."""
                junk = psum.tile([128, 512], f32, tag=tag, name=name)
                for i in range(n):
                    nc.tensor.matmul(
                        junk[:], wqk[:, 0, 0:128], wqk[:, i % KT, 0:512],
                        start=True, stop=True)

            warm_pe(12, "warmup", "at")

            def qk_half(m, half, tag):
                # m: 0,1 = q pair 0/1; 2,3 = k pair 0/1; half = 1024 cols
                dst = qt if m < 2 else kt
                pp = psum.tile([128, 1024], f32, tag=tag, name=f"qk{m}{half}")
                for k in range(KT):
                    for c2 in range(2):
                        o = 1024 * half + 512 * c2
                        nc.tensor.matmul(
                            pp[:, 512 * c2:512 * c2 + 512],
                            wqk[:, k, 128 * m:128 * m + 128],
                            xt[k][:, o:o + 512],
                            start=(k == 0), stop=(k == KT - 1))
                nc.vector.tensor_copy(
                    dst[:, m % 2, 1024 * half:1024 * half + 1024], pp[:])

            def v_proj(j, tag):
                vps = psum.tile([128, LCH], f32, tag=tag, name=f"v{j}")
                for k in range(KT):
                    nc.tensor.matmul(
                        vps[:], xt[k][:, 128 * j:128 * j + 128], wv[:, k, :],
                        start=(k == 0), stop=(k == KT - 1))
                nc.vector.tensor_copy(
                    v[:, j, :, 64:65], ones4[:].rearrange("p (h o) -> p h o", o=1))
                nc.vector.tensor_copy(
                    v[:, j, :, 0:64], vps[:].rearrange("p (h e) -> p h e", h=LHEADS))

            ag = {}
            ccin_last = [None]

            def stage_chunk(p, c, attps):
                """Copy the accumulators out of PSUM in one shot (frees the
                at/pp space immediately), then normalize from SBUF and ship
                both heads to the collective buffer in one DMA."""
                asb = work.tile([65, 2 * CHUNK], f32, tag="asb", bufs=2,
                                name=f"asb{p}{c}")
                nc.vector.tensor_copy(asb[:], attps[:])
                rcs = work.tile([64, 16], f32, tag="rcs", bufs=2,
                                name=f"rcs{p}{c}")
                nc.sync.dma_start(rcs[:], asb[64:65, :])
                rcr = work.tile([64, 16], f32, tag="rcr", bufs=2,
                                name=f"rcr{p}{c}")
                nc.vector.reciprocal(rcr[:], rcs[:])
                rc0 = work.tile([1, 2 * CHUNK], f32, tag="rc0", bufs=2,
                                name=f"rc0{p}{c}")
                nc.sync.dma_start(rc0[0:1, :], rcr[:])
                bc = work.tile([64, 2 * CHUNK], f32, tag="bc", bufs=2,
                               name=f"bc{p}{c}")
                nc.gpsimd.partition_broadcast(bc[:], rc0[0:1, :])
                ao = work.tile([64, 2 * CHUNK], f16, tag="ao", bufs=2,
                               name=f"ao{p}{c}")
                nc.vector.tensor_mul(ao[:, :], asb[0:64, :], bc[:, :])
                # both heads in one DMA: src col 512h+i -> dst row 64h+q
                if c == 3:
                    dst = cc_in3[p][:].rearrange("(h q) n -> q h n", h=2)
                    ccin_last[0] = nc.sync.dma_start(dst[:, :, :], ao[:, :])
                else:
                    dst = cc_in[c][:].rearrange("(h q) n -> q h n", h=2)
                    ccin_last[0] = nc.sync.dma_start(
                        dst[:, :, CHUNK * p:CHUNK * p + CHUNK], ao[:, :])

            def gather(g):
                nc.gpsimd.collective_compute(
                    "AllGather", mybir.AluOpType.bypass, replica_groups=RG,
                    ins=[cc_in[g][:]], outs=[cc_out[g][:]])

            def gather3(p):
                nc.gpsimd.collective_compute(
                    "AllGather", mybir.AluOpType.bypass, replica_groups=RG,
                    ins=[cc_in3[p][:]], outs=[cc_out3[p][:]])

            def prefetch(g):
                # Pin behind the newest staging DMA so the scheduler cannot
                # hoist the AllGather wait ahead of attention staging.
                for r in range(GROUP):
                    t = agp.tile([128, 2 * CHUNK], f16, name=f"ag{g}{r}")
                    dma = nc.sync.dma_start(
                        t[:], cc_out[g][128 * r:128 * r + 128, :])
                    if ccin_last[0] is not None:
                        tile.add_dep_helper(
                            dma.ins, ccin_last[0].ins, sync=True,
                            reason="gather prefetch after staging")
                    ag[(g, r)] = t

            def prefetch3(p):
                for r in range(GROUP):
                    t = agp.tile([128, CHUNK], f16, name=f"ag3{p}{r}")
                    dma = nc.sync.dma_start(
                        t[:], cc_out3[p][128 * r:128 * r + 128, :])
                    if ccin_last[0] is not None:
                        tile.add_dep_helper(
                            dma.ins, ccin_last[0].ins, sync=True,
                            reason="gather prefetch after staging")
                    ag[(3, p, r)] = t

            def out_ct(g, ct, tag):
                pp = psum.tile([128, CHUNK], f32, tag=tag, name=f"op{g}{ct}")
                for k in range(KT):
                    rhs = (ag[(3, k // 4, k % 4)][:, :] if g == 3 else
                           ag[(g, k % 4)][:, CHUNK * (k // 4):
                                          CHUNK * (k // 4) + CHUNK])
                    nc.tensor.matmul(
                        pp[:], wo[:, k, 128 * ct:128 * ct + 128], rhs,
                        start=(k == 0), stop=(k == KT - 1))
                ot = agp.tile([128, CHUNK], f16, tag=f"ot{ct}", bufs=2,
                              name=f"ot{g}{ct}")
                nc.scalar.copy(ot[:], pp[:])
                nc.sync.dma_start(
                    outT[128 * ct:128 * ct + 128,
                         CHUNK * g:CHUNK * g + CHUNK], ot[:])

            # Filler queue: self-contained closures (own PSUM tile, tag
            # passed at drain time = opposite of the running chunk's attps).
            FILL = deque()

            def drain_all():
                while FILL:
                    FILL.popleft()("pp")   # between chunks: either tag works

            def attn_chunk(p, c, gc):
                opp = "pp" if gc % 2 == 0 else "at"
                q0 = CHUNK * c
                nj = 4 * c + 4
                attps = psum.tile([65, 2 * CHUNK], f32,
                                  tag=("at" if gc % 2 == 0 else "pp"),
                                  name=f"att{p}{c}")
                for j in range(nj):
                    qs = max(q0, 128 * j)
                    n = q0 + CHUNK - qs
                    off = qs - q0
                    sc = psum.tile([128, 1024], f32, tag="sc", bufs=2,
                                   name=f"sc{p}{c}{j}")
                    for h in range(2):
                        pb = 64 * h
                        nc.tensor.matmul(
                            sc[:, 512 * h:512 * h + n],
                            kt[pb:pb + 64, p, 128 * j:128 * j + 128],
                            qt[pb:pb + 64, p, qs:qs + n],
                            start=True, stop=True)
                    ex = work.tile([128, 1024], f16, tag="ex", bufs=3,
                                   name=f"ex{p}{c}{j}")
                    if n == CHUNK:
                        nc.scalar.activation(ex[:, :], sc[:, :], Exp)
                    else:
                        nc.scalar.activation(
                            ex[:].rearrange("q (t x) -> q t x", t=2)[:, :, 0:n],
                            sc[:].rearrange("q (t x) -> q t x", t=2)[:, :, 0:n],
                            Exp)
                    if qs == 128 * j:  # diagonal tile: causal mask
                        for h in range(2):
                            nc.vector.tensor_mul(
                                ex[:, 512 * h:512 * h + 128],
                                ex[:, 512 * h:512 * h + 128], mask_sb[:])
                    for h in range(2):
                        nc.tensor.matmul(
                            attps[:, 512 * h + off:512 * h + CHUNK],
                            v[:, j, 2 * p + h, :],
                            ex[:, 512 * h:512 * h + n],
                            start=(j == 0), stop=(j == nj - 1))
                    if FILL and j < nj - 1 and (j >= 3 or (nj <= 6 and j >= 1)):
                        FILL.popleft()(opp)
                stage_chunk(p, c, attps)

            def qk_fillers(m, half):
                """q/k half as two filler closures sharing one PSUM group;
                drained on consecutive j-slots of a single chunk."""
                state = {}

                def part(ks):
                    def go(tag):
                        if "pp" not in state:
                            state["pp"] = psum.tile(
                                [128, 1024], f32, tag=tag, name=f"qk{m}{half}")
                        pp = state["pp"]
                        for k in ks:
                            for c2 in range(2):
                                o = 1024 * half + 512 * c2
                                nc.tensor.matmul(
                                    pp[:, 512 * c2:512 * c2 + 512],
                                    wqk[:, k, 128 * m:128 * m + 128],
                                    xt[k][:, o:o + 512],
                                    start=(k == 0), stop=(k == KT - 1))
                        if ks[-1] == KT - 1:
                            dst = qt if m < 2 else kt
                            nc.vector.tensor_copy(
                                dst[:, m % 2, 1024 * half:1024 * half + 1024],
                                pp[:])
                    return go
                return [part(range(0, 4)), part(range(4, 8))]

            def qk_quarter(m, half, c2, tag):
                """512 columns of a q/k projection: one PSUM accumulation
                group, fully self-contained (chunk 0/1 only needs the first
                512 columns of each pair's q and k)."""
                pp = psum.tile([128, 512], f32, tag=tag, name=f"qkq{m}{half}{c2}")
                o = 1024 * half + 512 * c2
                for k in range(KT):
                    nc.tensor.matmul(
                        pp[:], wqk[:, k, 128 * m:128 * m + 128],
                        xt[k][:, o:o + 512],
                        start=(k == 0), stop=(k == KT - 1))
                dst = qt if m < 2 else kt
                nc.vector.tensor_copy(dst[:, m % 2, o:o + 512], pp[:])

            # ---------------- schedule ----------------
            qk_quarter(0, 0, 0, "at")  # q/k pair-0 cols 0-511: all chunk 0
            qk_quarter(2, 0, 0, "pp")  # needs
            v_proj(0, "at")
            v_proj(1, "pp")
            v_proj(2, "at")
            v_proj(3, "pp")
            FILL.append(lambda t: qk_quarter(0, 0, 1, t))
            FILL.append(lambda t: qk_quarter(2, 0, 1, t))
            attn_chunk(0, 0, 0)        # attps at
            qk_quarter(1, 0, 0, "pp")
            qk_quarter(3, 0, 0, "pp")
            FILL.append(lambda t: qk_quarter(1, 0, 1, t))
            FILL.append(lambda t: qk_quarter(3, 0, 1, t))
            for j in range(4, 8):
                FILL.append(lambda t, j=j: v_proj(j, t))
            attn_chunk(1, 0, 1)        # attps pp
            gather(0)
            # v4-7 drain just-in-time inside gc2 (v_j lands before av_j);
            # pair-0 q/k cols 1024-2047 drain inside gc3 (needed by gc4)
            FILL.extend(qk_fillers(0, 1))
            FILL.extend(qk_fillers(2, 1))
            attn_chunk(0, 1, 2)        # at
            attn_chunk(1, 1, 3)        # pp
            gather(1)
            prefetch(0)
            drain_all()
            for j in range(8, 12):     # v8-11 + pair-1 h1: inside gc4
                FILL.append(lambda t, j=j: v_proj(j, t))
            FILL.extend(qk_fillers(1, 1))
            FILL.extend(qk_fillers(3, 1))
            attn_chunk(0, 2, 4)        # at
            drain_all()                # q/k half1 (pair 1) before chunk(1,2)
            for j in range(12, 16):
                FILL.append(lambda t, j=j: v_proj(j, t))
            FILL.append(lambda t: out_ct(0, 0, t))
            FILL.append(lambda t: out_ct(0, 1, t))
            attn_chunk(1, 2, 5)        # pp
            gather(2)
            prefetch(1)
            drain_all()                # v12-15 before chunk 3
            FILL.append(lambda t: out_ct(1, 0, t))
            FILL.append(lambda t: out_ct(1, 1, t))
            attn_chunk(0, 3, 6)        # at
            gather3(0)
            prefetch(2)
            attn_chunk(1, 3, 7)        # pp
            gather3(1)
            drain_all()
            out_ct(2, 0, "at")         # AG2 landed during chunk 3
            out_ct(2, 1, "at")
            warm_pe(24, "tailwarm", "pp")  # keep PE warm while gather3 flies
            prefetch3(0)
            prefetch3(1)
            out_ct(3, 0, "pp")
            out_ct(3, 1, "at")

    nc.compile()
    return nc


def _gather_perm():
    """d-channel permutation matching the AllGather layout."""
    perm = []
    for p in range(PAIRS):
        for r in range(GROUP):
            for h in range(2):
                head = 4 * r + 2 * p + h
                perm.extend(range(HD * head, HD * head + HD))
    return np.array(perm)


def _shard_inputs(x, Wq, Wk, Wv, Wo):
    x = np.asarray(x, dtype=np.float32)
    Wq = np.asarray(Wq, dtype=np.float32)
    Wk = np.asarray(Wk, dtype=np.float32)
    Wv = np.asarray(Wv, dtype=np.float32)
    Wo = np.asarray(Wo, dtype=np.float32)
    mask = np.triu(np.ones((128, 128), dtype=np.float16))
    perm = _gather_perm()
    in_maps = []
    for c in range(NCORES):
        b, g = c // GROUP, c % GROUP
        rows = slice(LCH * g, LCH * g + LCH)
        in_maps.append({
            "xT": _f16(x[b].T),
            "wqkT": _f16(np.concatenate([Wq[rows] / 8.0, Wk[rows]], axis=0).T),
            "wvT": _f16(Wv[rows].T),
            "woT": _f16(Wo[rows].T[perm, :]),
            "mask": mask,
        })
    return in_maps


def kernel(x, Wq, Wk, Wv, Wo):
    from concourse.bass_utils import run_bass_kernel_spmd

    if "nc" not in _CACHE:
        _CACHE["nc"] = _build()
    nc = _CACHE["nc"]
    in_maps = _shard_inputs(x, Wq, Wk, Wv, Wo)
    res = run_bass_kernel_spmd(nc, in_maps, core_ids=list(range(NCORES)))
    _CACHE["last_results"] = res
    out = np.empty((B, S, D), dtype=np.float32)
    for c in range(NCORES):
        b, g = c // GROUP, c % GROUP
        out[b][:, LCH * g:LCH * g + LCH] = \
            res.results[c]["outT"].T.astype(np.float32)
    return out


# revision 27
# speedup vs baseline: 1.1133x; 1.0131x over previous
"""Causal self-attention on 8 Trainium2 NeuronCores.

Sharding (data + head parallel): core c handles batch b = c // 4 and the
4 heads [4g, 4g+4) where g = c % 4.  Each core projects q/k/v for its
heads (weights pre-sliced + pre-transposed on host), runs causal
attention, then the 4 cores of each batch AllGather the per-head
attention outputs (hd-major fp16) and each computes a disjoint
256-channel column slice of the output projection.

Schedule notes (v4):
- The attention inner loop is exp(ACT)-paced (~1us per 128-key step);
  projection work (q/k second halves, v tiles, output projection) is
  emitted as small self-contained "filler" closures drained one per
  j-step inside the attention loops, so the in-order PE queue never
  parks a multi-us block in front of the next score matmul.
- One AllGather per 512-column chunk (both head pairs) fires as soon as
  the chunk is staged on both pairs; gather g's SBUF prefetch is issued
  about one chunk later, and its out_proj slice runs as filler inside a
  later chunk.  Only the final chunk's gather + out_proj sit in the
  tail.
- Dummy matmuls on the first-loaded weight tile warm the PE clock (HAM
  K=8/8) while x streams in, and again while the last gather flies so
  the tail out_proj runs at 2.4GHz.
- Softmax normalization: accumulators leave PSUM in one copy (frees the
  at/pp space immediately); the 1024 denominators are reshaped across
  64 partitions by DMA, inverted (DVE reciprocal is serial per
  partition), reshaped back, broadcast once to [64,1024], and applied
  with a single multiply; one DMA ships both heads to the collective
  buffer.
- A tiny warmup AllGather at kernel start absorbs CC-stream ramp and
  cross-core launch skew.
- fp16 data path, fp32 PSUM accumulation, fp16 output (absmax ~4).
- PSUM budget (8 banks): score tile 2 banks x 2 bufs; "at"/"pp" spaces
  2 banks each, alternating between attention accumulators and
  filler projection tiles (fillers always use the opposite space of
  the chunk they are drained into).

Layouts per core:
  xT    (1024, 2048)  x[b].T                       (d on partitions)
  wqkT  (1024, 512)   [ (Wq[rows]/8).T | Wk[rows].T ]
  wvT   (1024, 256)   Wv[rows].T
  woT   (1024, 256)   Wo[rows].T with rows permuted to the AllGather
                      order: [pair p=0: rank r: heads 4r,4r+1] then
                      [pair p=1: rank r: heads 4r+2,4r+3]
  mask  (128, 128)    upper-triangular ones (k <= q)
  outT  (256, 2048)   out[b][:, cols].T  (fp16)
"""

from collections import deque

import numpy as np

B, S, D, H = 2, 2048, 1024, 16
HD = D // H              # 64
NCORES = 8
GROUP = 4                # cores per batch
LHEADS = 4               # heads per core
LCH = LHEADS * HD        # 256 local channels
KT = D // 128            # 8 contraction tiles
ST = S // 128            # 16 sequence tiles
PAIRS = 2                # head pairs per core
CHUNK = 512              # q columns per attention pass / gather block
NCH = S // CHUNK         # 4

_CACHE = {}


def _f16(a):
    return np.ascontiguousarray(a, dtype=np.float16)


def _build():
    import concourse.bacc as bacc
    import concourse.mybir as mybir
    import concourse.tile as tile

    f32 = mybir.dt.float32
    f16 = mybir.dt.float16
    Exp = mybir.ActivationFunctionType.Exp

    nc = bacc.Bacc(num_devices=NCORES)
    xT = nc.dram_tensor("xT", [D, S], f16, kind="ExternalInput")
    wqkT = nc.dram_tensor("wqkT", [D, 2 * LCH], f16, kind="ExternalInput")
    wvT = nc.dram_tensor("wvT", [D, LCH], f16, kind="ExternalInput")
    woT = nc.dram_tensor("woT", [D, LCH], f16, kind="ExternalInput")
    mask = nc.dram_tensor("mask", [128, 128], f16, kind="ExternalInput")
    outT = nc.dram_tensor("outT", [LCH, S], f16, kind="ExternalOutput")

    RG = [[0, 1, 2, 3], [4, 5, 6, 7]]

    with tile.TileContext(nc, num_cores=NCORES) as tc:
        with (
            tc.tile_pool(name="const", bufs=1) as const,
            tc.tile_pool(name="qkv", bufs=1) as qkv,
            tc.tile_pool(name="psum", bufs=1, space="PSUM") as psum,
            tc.tile_pool(name="dram", bufs=1, space="DRAM") as dram,
            tc.tile_pool(name="work", bufs=1) as work,
            tc.tile_pool(name="proj", bufs=1) as projp,
            tc.tile_pool(name="agp", bufs=1) as agp,
        ):
            # warmup collective first: absorbs CC-stream ramp + launch skew
            warm_in = dram.tile([128, 8], f16, name="warm_in")
            warm_out = dram.tile([GROUP * 128, 8], f16, name="warm_out")
            nc.sync.dma_start(warm_in[:], mask[:, 0:8])
            nc.gpsimd.collective_compute(
                "AllGather", mybir.AluOpType.bypass, replica_groups=RG,
                ins=[warm_in[:]], outs=[warm_out[:]])

            # chunks 0-2 gather both pairs at once; chunk 3 gathers per pair
            # so the tail only waits on the last 128KB collective
            cc_in = [dram.tile([128, 2 * CHUNK], f16, name=f"ccin{g}")
                     for g in range(NCH - 1)]
            cc_out = [dram.tile([GROUP * 128, 2 * CHUNK], f16, name=f"ccout{g}")
                      for g in range(NCH - 1)]
            cc_in3 = [dram.tile([128, CHUNK], f16, name=f"ccin3{p}")
                      for p in range(PAIRS)]
            cc_out3 = [dram.tile([GROUP * 128, CHUNK], f16, name=f"ccout3{p}")
                       for p in range(PAIRS)]

            mask_sb = const.tile([128, 128], f16)
            ones4 = const.tile([128, LHEADS], f32)
            nc.vector.memset(ones4[:], 1.0)

            qt = qkv.tile([128, PAIRS, S], f16)
            kt = qkv.tile([128, PAIRS, S], f16)
            v = qkv.tile([128, ST, LHEADS, 65], f16)

            # ---------------- input loads ----------------
            # spread across the two hardware DGE rings (sync + scalar) so
            # per-ring serialization + coarse semaphores don't delay the
            # first projection matmuls
            nc.sync.dma_start(mask_sb[:], mask[:])
            wqk = projp.tile([128, KT, 2 * LCH], f16)
            nc.scalar.dma_start(wqk[:], wqkT[:].rearrange("(k p) n -> p k n", p=128))
            xt = []
            for k in range(KT):
                tx = projp.tile([128, S], f16, name=f"xt{k}")
                eng = nc.sync if k % 2 == 0 else nc.scalar
                eng.dma_start(tx[:], xT[128 * k:128 * k + 128, :])
                xt.append(tx)
            wv = projp.tile([128, KT, LCH], f16)
            nc.sync.dma_start(wv[:], wvT[:].rearrange("(k p) n -> p k n", p=128))
            wo = projp.tile([128, KT, LCH], f16)
            nc.scalar.dma_start(wo[:], woT[:].rearrange("(k p) n -> p k n", p=128))

            def warm_pe(n, name, tag, wide=True):
                """Dummy matmuls: keep the PE HAM warm while it would
                otherwise idle (startup x-load, tail gather wait).  The
                narrow variant runs on the 32KB mask tile, which lands
                ~8us before the first big weight tile."""
                src = wqk[:, 0, 0:512] if wide else mask_sb[:, 0:128]
                ncols = 512 if wide else 128
                junk = psum.tile([128, ncols], f32, tag=tag, name=name)
                for i in range(n):
                    nc.tensor.matmul(
                        junk[:], mask_sb[:, 0:128] if not wide else
                        wqk[:, 0, 0:128], src,
                        start=True, stop=True)

            warm_pe(40, "warmup", "at", wide=False)

            def qk_half(m, half, tag):
                # m: 0,1 = q pair 0/1; 2,3 = k pair 0/1; half = 1024 cols
                dst = qt if m < 2 else kt
                pp = psum.tile([128, 1024], f32, tag=tag, name=f"qk{m}{half}")
                for k in range(KT):
                    for c2 in range(2):
                        o = 1024 * half + 512 * c2
                        nc.tensor.matmul(
                            pp[:, 512 * c2:512 * c2 + 512],
                            wqk[:, k, 128 * m:128 * m + 128],
                            xt[k][:, o:o + 512],
                            start=(k == 0), stop=(k == KT - 1))
                nc.vector.tensor_copy(
                    dst[:, m % 2, 1024 * half:1024 * half + 1024], pp[:])

            def v_proj(j, tag):
                vps = psum.tile([128, LCH], f32, tag=tag, name=f"v{j}")
                for k in range(KT):
                    nc.tensor.matmul(
                        vps[:], xt[k][:, 128 * j:128 * j + 128], wv[:, k, :],
                        start=(k == 0), stop=(k == KT - 1))
                nc.vector.tensor_copy(
                    v[:, j, :, 64:65], ones4[:].rearrange("p (h o) -> p h o", o=1))
                nc.vector.tensor_copy(
                    v[:, j, :, 0:64], vps[:].rearrange("p (h e) -> p h e", h=LHEADS))

            ag = {}
            ccin_last = [None]

            def stage_chunk(p, c, attps):
                """Copy the accumulators out of PSUM in one shot (frees the
                at/pp space immediately), then normalize from SBUF and ship
                both heads to the collective buffer in one DMA."""
                asb = work.tile([65, 2 * CHUNK], f32, tag="asb", bufs=2,
                                name=f"asb{p}{c}")
                nc.vector.tensor_copy(asb[:], attps[:])
                rcs = work.tile([64, 16], f32, tag="rcs", bufs=2,
                                name=f"rcs{p}{c}")
                nc.sync.dma_start(rcs[:], asb[64:65, :])
                rcr = work.tile([64, 16], f32, tag="rcr", bufs=2,
                                name=f"rcr{p}{c}")
                nc.vector.reciprocal(rcr[:], rcs[:])
                rc0 = work.tile([1, 2 * CHUNK], f32, tag="rc0", bufs=2,
                                name=f"rc0{p}{c}")
                nc.sync.dma_start(rc0[0:1, :], rcr[:])
                bc = work.tile([64, 2 * CHUNK], f32, tag="bc", bufs=2,
                               name=f"bc{p}{c}")
                nc.gpsimd.partition_broadcast(bc[:], rc0[0:1, :])
                ao = work.tile([64, 2 * CHUNK], f16, tag="ao", bufs=2,
                               name=f"ao{p}{c}")
                nc.vector.tensor_mul(ao[:, :], asb[0:64, :], bc[:, :])
                # both heads in one DMA: src col 512h+i -> dst row 64h+q
                if c == 3:
                    dst = cc_in3[p][:].rearrange("(h q) n -> q h n", h=2)
                    ccin_last[0] = nc.sync.dma_start(dst[:, :, :], ao[:, :])
                else:
                    dst = cc_in[c][:].rearrange("(h q) n -> q h n", h=2)
                    ccin_last[0] = nc.sync.dma_start(
                        dst[:, :, CHUNK * p:CHUNK * p + CHUNK], ao[:, :])

            def gather(g):
                nc.gpsimd.collective_compute(
                    "AllGather", mybir.AluOpType.bypass, replica_groups=RG,
                    ins=[cc_in[g][:]], outs=[cc_out[g][:]])

            def gather3(p):
                nc.gpsimd.collective_compute(
                    "AllGather", mybir.AluOpType.bypass, replica_groups=RG,
                    ins=[cc_in3[p][:]], outs=[cc_out3[p][:]])

            def prefetch(g):
                # Pin behind the newest staging DMA so the scheduler cannot
                # hoist the AllGather wait ahead of attention staging.
                for r in range(GROUP):
                    t = agp.tile([128, 2 * CHUNK], f16, name=f"ag{g}{r}")
                    dma = nc.sync.dma_start(
                        t[:], cc_out[g][128 * r:128 * r + 128, :])
                    if ccin_last[0] is not None:
                        tile.add_dep_helper(
                            dma.ins, ccin_last[0].ins, sync=True,
                            reason="gather prefetch after staging")
                    ag[(g, r)] = t

            def prefetch3(p):
                for r in range(GROUP):
                    t = agp.tile([128, CHUNK], f16, name=f"ag3{p}{r}")
                    dma = nc.sync.dma_start(
                        t[:], cc_out3[p][128 * r:128 * r + 128, :])
                    if ccin_last[0] is not None:
                        tile.add_dep_helper(
                            dma.ins, ccin_last[0].ins, sync=True,
                            reason="gather prefetch after staging")
                    ag[(3, p, r)] = t

            def out_ct(g, ct, tag):
                pp = psum.tile([128, CHUNK], f32, tag=tag, name=f"op{g}{ct}")
                for k in range(KT):
                    rhs = (ag[(3, k // 4, k % 4)][:, :] if g == 3 else
                           ag[(g, k % 4)][:, CHUNK * (k // 4):
                                          CHUNK * (k // 4) + CHUNK])
                    nc.tensor.matmul(
                        pp[:], wo[:, k, 128 * ct:128 * ct + 128], rhs,
                        start=(k == 0), stop=(k == KT - 1))
                ot = agp.tile([128, CHUNK], f16, tag=f"ot{ct}", bufs=2,
                              name=f"ot{g}{ct}")
                nc.scalar.copy(ot[:], pp[:])
                nc.sync.dma_start(
                    outT[128 * ct:128 * ct + 128,
                         CHUNK * g:CHUNK * g + CHUNK], ot[:])

            # Filler queue: self-contained closures (own PSUM tile, tag
            # passed at drain time = opposite of the running chunk's attps).
            FILL = deque()

            def drain_all():
                while FILL:
                    FILL.popleft()("pp")   # between chunks: either tag works

            def attn_chunk(p, c, gc):
                opp = "pp" if gc % 2 == 0 else "at"
                q0 = CHUNK * c
                nj = 4 * c + 4
                attps = psum.tile([65, 2 * CHUNK], f32,
                                  tag=("at" if gc % 2 == 0 else "pp"),
                                  name=f"att{p}{c}")
                for j in range(nj):
                    qs = max(q0, 128 * j)
                    n = q0 + CHUNK - qs
                    off = qs - q0
                    sc = psum.tile([128, 1024], f32, tag="sc", bufs=2,
                                   name=f"sc{p}{c}{j}")
                    for h in range(2):
                        pb = 64 * h
                        nc.tensor.matmul(
                            sc[:, 512 * h:512 * h + n],
                            kt[pb:pb + 64, p, 128 * j:128 * j + 128],
                            qt[pb:pb + 64, p, qs:qs + n],
                            start=True, stop=True)
                    ex = work.tile([128, 1024], f16, tag="ex", bufs=3,
                                   name=f"ex{p}{c}{j}")
                    if n == CHUNK:
                        nc.scalar.activation(ex[:, :], sc[:, :], Exp)
                    else:
                        nc.scalar.activation(
                            ex[:].rearrange("q (t x) -> q t x", t=2)[:, :, 0:n],
                            sc[:].rearrange("q (t x) -> q t x", t=2)[:, :, 0:n],
                            Exp)
                    if qs == 128 * j:  # diagonal tile: causal mask
                        for h in range(2):
                            nc.vector.tensor_mul(
                                ex[:, 512 * h:512 * h + 128],
                                ex[:, 512 * h:512 * h + 128], mask_sb[:])
                    for h in range(2):
                        nc.tensor.matmul(
                            attps[:, 512 * h + off:512 * h + CHUNK],
                            v[:, j, 2 * p + h, :],
                            ex[:, 512 * h:512 * h + n],
                            start=(j == 0), stop=(j == nj - 1))
                    if FILL and j < nj - 1 and (j >= 3 or (nj <= 6 and j >= 1)):
                        FILL.popleft()(opp)
                stage_chunk(p, c, attps)

            def qk_fillers(m, half):
                """q/k half as two filler closures sharing one PSUM group;
                drained on consecutive j-slots of a single chunk."""
                state = {}

                def part(ks):
                    def go(tag):
                        if "pp" not in state:
                            state["pp"] = psum.tile(
                                [128, 1024], f32, tag=tag, name=f"qk{m}{half}")
                        pp = state["pp"]
                        for k in ks:
                            for c2 in range(2):
                                o = 1024 * half + 512 * c2
                                nc.tensor.matmul(
                                    pp[:, 512 * c2:512 * c2 + 512],
                                    wqk[:, k, 128 * m:128 * m + 128],
                                    xt[k][:, o:o + 512],
                                    start=(k == 0), stop=(k == KT - 1))
                        if ks[-1] == KT - 1:
                            dst = qt if m < 2 else kt
                            nc.vector.tensor_copy(
                                dst[:, m % 2, 1024 * half:1024 * half + 1024],
                                pp[:])
                    return go
                return [part(range(0, 4)), part(range(4, 8))]

            def qk_quarter(m, half, c2, tag):
                """512 columns of a q/k projection: one PSUM accumulation
                group, fully self-contained (chunk 0/1 only needs the first
                512 columns of each pair's q and k)."""
                pp = psum.tile([128, 512], f32, tag=tag, name=f"qkq{m}{half}{c2}")
                o = 1024 * half + 512 * c2
                for k in range(KT):
                    nc.tensor.matmul(
                        pp[:], wqk[:, k, 128 * m:128 * m + 128],
                        xt[k][:, o:o + 512],
                        start=(k == 0), stop=(k == KT - 1))
                dst = qt if m < 2 else kt
                nc.vector.tensor_copy(dst[:, m % 2, o:o + 512], pp[:])

            # ---------------- schedule ----------------
            qk_quarter(0, 0, 0, "at")  # q/k pair-0 cols 0-511: all chunk 0
            qk_quarter(2, 0, 0, "pp")  # needs
            v_proj(0, "at")
            v_proj(1, "pp")
            v_proj(2, "at")
            v_proj(3, "pp")
            FILL.append(lambda t: qk_quarter(0, 0, 1, t))
            FILL.append(lambda t: qk_quarter(2, 0, 1, t))
            attn_chunk(0, 0, 0)        # attps at
            qk_quarter(1, 0, 0, "pp")
            qk_quarter(3, 0, 0, "pp")
            FILL.append(lambda t: qk_quarter(1, 0, 1, t))
            FILL.append(lambda t: qk_quarter(3, 0, 1, t))
            for j in range(4, 8):
                FILL.append(lambda t, j=j: v_proj(j, t))
            attn_chunk(1, 0, 1)        # attps pp
            gather(0)
            # v4-7 drain just-in-time inside gc2 (v_j lands before av_j);
            # pair-0 q/k cols 1024-2047 drain inside gc3 (needed by gc4)
            FILL.extend(qk_fillers(0, 1))
            FILL.extend(qk_fillers(2, 1))
            attn_chunk(0, 1, 2)        # at
            attn_chunk(1, 1, 3)        # pp
            gather(1)
            prefetch(0)
            drain_all()
            for j in range(8, 12):     # v8-11 + pair-1 h1: inside gc4
                FILL.append(lambda t, j=j: v_proj(j, t))
            FILL.extend(qk_fillers(1, 1))
            FILL.extend(qk_fillers(3, 1))
            attn_chunk(0, 2, 4)        # at
            drain_all()                # q/k half1 (pair 1) before chunk(1,2)
            for j in range(12, 16):
                FILL.append(lambda t, j=j: v_proj(j, t))
            FILL.append(lambda t: out_ct(0, 0, t))
            FILL.append(lambda t: out_ct(0, 1, t))
            attn_chunk(1, 2, 5)        # pp
            gather(2)
            prefetch(1)
            drain_all()                # v12-15 before chunk 3
            FILL.append(lambda t: out_ct(1, 0, t))
            FILL.append(lambda t: out_ct(1, 1, t))
            attn_chunk(0, 3, 6)        # at
            gather3(0)
            prefetch(2)
            attn_chunk(1, 3, 7)        # pp
            gather3(1)
            drain_all()
            prefetch3(0)               # AG3a landed during chunk (1,3)
            out_ct(2, 0, "at")         # AG2 landed during chunk 3
            out_ct(2, 1, "at")
            # out_ct(3, ct0) split around the last gather: the pair-0 half
            # accumulates from AG3a data now; junk matmuls keep the PE warm
            # while AG3b flies; then the pair-1 half closes the group.
            op30 = psum.tile([128, CHUNK], f32, tag="pp", name="op30")
            for k in range(4):
                nc.tensor.matmul(
                    op30[:], wo[:, k, 0:128], ag[(3, 0, k)][:, :],
                    start=(k == 0), stop=False)
            warm_pe(32, "tailwarm", "at")
            prefetch3(1)
            for k in range(4, KT):
                nc.tensor.matmul(
                    op30[:], wo[:, k, 0:128], ag[(3, 1, k - 4)][:, :],
                    start=False, stop=(k == KT - 1))
            ot30 = agp.tile([128, CHUNK], f16, tag="ot0", bufs=2, name="ot30")
            nc.scalar.copy(ot30[:], op30[:])
            nc.sync.dma_start(outT[0:128, CHUNK * 3:CHUNK * 4], ot30[:])
            out_ct(3, 1, "at")

    nc.compile()
    return nc


def _gather_perm():
    """d-channel permutation matching the AllGather layout."""
    perm = []
    for p in range(PAIRS):
        for r in range(GROUP):
            for h in range(2):
                head = 4 * r + 2 * p + h
                perm.extend(range(HD * head, HD * head + HD))
    return np.array(perm)


def _shard_inputs(x, Wq, Wk, Wv, Wo):
    x = np.asarray(x, dtype=np.float32)
    Wq = np.asarray(Wq, dtype=np.float32)
    Wk = np.asarray(Wk, dtype=np.float32)
    Wv = np.asarray(Wv, dtype=np.float32)
    Wo = np.asarray(Wo, dtype=np.float32)
    mask = np.triu(np.ones((128, 128), dtype=np.float16))
    perm = _gather_perm()
    in_maps = []
    for c in range(NCORES):
        b, g = c // GROUP, c % GROUP
        rows = slice(LCH * g, LCH * g + LCH)
        in_maps.append({
            "xT": _f16(x[b].T),
            "wqkT": _f16(np.concatenate([Wq[rows] / 8.0, Wk[rows]], axis=0).T),
            "wvT": _f16(Wv[rows].T),
            "woT": _f16(Wo[rows].T[perm, :]),
            "mask": mask,
        })
    return in_maps


def kernel(x, Wq, Wk, Wv, Wo):
    from concourse.bass_utils import run_bass_kernel_spmd

    if "nc" not in _CACHE:
        _CACHE["nc"] = _build()
    nc = _CACHE["nc"]
    in_maps = _shard_inputs(x, Wq, Wk, Wv, Wo)
    res = run_bass_kernel_spmd(nc, in_maps, core_ids=list(range(NCORES)))
    _CACHE["last_results"] = res
    out = np.empty((B, S, D), dtype=np.float32)
    for c in range(NCORES):
        b, g = c // GROUP, c % GROUP
        out[b][:, LCH * g:LCH * g + LCH] = \
            res.results[c]["outT"].T.astype(np.float32)
    return out
